# revision 8
# baseline (speedup 1.0000x reference)
"""Trainium2 Bass kernel for GQA attention (B=2, S=2048, DIM=2048, H=16, KV=8,
HD=128) with RoPE + causal mask + output projection.

Sharding: 8-way tensor parallelism over heads. Core c computes q heads
{2c, 2c+1} and kv head c end-to-end (QKV projection, RoPE, causal attention),
then multiplies its own attention output by its ROW slice of wo (rows
256c:256(c+1)), producing a partial full-width output; a ReduceScatter sums
partials across cores and hands core c the output-feature rows
[256c:256(c+1)], which the host transposes/concats. The PE never waits on a
collective: all matmul work is local.

v4 pipeline notes (vs v3 baseline):
- softmax denominator accumulated on the PE (ones-vector matmul per j block,
  PSUM-accumulated alongside the AV matmul) instead of a DVE add chain; kills
  ~110us of DVE work and the per-chunk PE stalls waiting on it.
- projection PSUM windows are evicted to SBUF bf16 by the ACT engine
  immediately; RoPE runs SBUF->SBUF on the DVE in bf16 off the critical path
  (PSUM banks free ~2us after each window instead of ~12us).
- output projection computes per-core partials from local SBUF activations
  (no AllGather, no gather loads); partial tiles are cast/stored per
  128-row block and ReduceScattered straight into the output tensor.
- wo partial stores ride the DVE DMA queue; xt stream keeps the sync queue;
  weights/collectives ride the gpsimd queue. wq alone loads first on sync.
- finalize: reciprocal_approx_fast on [1,512] denominators, broadcast via
  ones-row matmul, ACT copy to bf16, single DVE multiply per head.

Layout tricks (unchanged from v3):
- everything computed transposed (feature dim on SBUF partitions); only
  on-device transposes are 16 PE transposes per batch for v.
- RoPE interleaved pairs handled by permuting wq/wk columns on the host to
  [evens, odds] per head; q/k permuted consistently so dot products are
  unchanged; v / wo stay unpermuted.
- softmax in scoresT layout (keys on partitions): no max subtraction (scores
  are O(5)), causal mask as a -30 additive bias accumulated by the PE
  (identity matmul) on diagonal blocks only.
- matmuls bf16 (fp32 accumulate); 1/sqrt(HD) folded into wq.
"""

import sys

if "/opt/trn_rl_repo" not in sys.path:
    sys.path.insert(0, "/opt/trn_rl_repo")

import numpy as np
import ml_dtypes

B, S, DIM = 2, 2048, 2048
H, KV, HD = 16, 8, 128
NC = 8
NS = B * S            # 4096 flattened (b, s) rows
P = 128
MB = DIM // P         # 16 contraction blocks for the projections
BF = ml_dtypes.bfloat16

_cache: dict = {}


def _build(debug=False):
    import concourse.bass as bass
    import concourse.mybir as mybir
    import concourse.tile as tile
    from concourse import bacc
    from concourse.masks import make_identity

    dt = mybir.dt
    f32, bf16 = dt.float32, dt.bfloat16
    Exp = mybir.ActivationFunctionType.Exp

    nc = bacc.Bacc("TRN2", debug=False, target_bir_lowering=False, num_devices=NC)

    # x^T arrives pre-tiled as [m_block, window, 128, 512] so every
    # projection-stream DMA is one contiguous 128KB block
    xT_h = nc.dram_tensor("xT", (MB, 8, P, 512), bf16, kind="ExternalInput").ap()
    # weights arrive pre-tiled as [mi=128, mb*d] so their DMAs are contiguous
    wq_h = nc.dram_tensor("wq_c", (P, MB * 256), bf16, kind="ExternalInput").ap()
    wk_h = nc.dram_tensor("wk_c", (P, MB * HD), bf16, kind="ExternalInput").ap()
    wv_h = nc.dram_tensor("wv_c", (P, MB * HD), bf16, kind="ExternalInput").ap()
    # wo row-slice for this core: [d=128, h=2, ofb=16, of=128]
    wo_h = nc.dram_tensor("wo_c", (P, 2 * MB * P), bf16, kind="ExternalInput").ap()
    cos_h = nc.dram_tensor("cosT", (64, NS), bf16, kind="ExternalInput").ap()
    sin_h = nc.dram_tensor("sinT", (64, NS), bf16, kind="ExternalInput").ap()
    mskb_h = nc.dram_tensor("maskb", (P, 4 * 512), bf16, kind="ExternalInput").ap()
    # per-(b,t) chunk of this core's 256 output-feature rows, from ReduceScatter
    out_h = nc.dram_tensor("outT", (B * 4, 256, 512), bf16,
                           kind="ExternalOutput").ap()

    with tile.TileContext(nc) as tc:
        with (
            tc.tile_pool(name="const", bufs=1) as const,
            tc.tile_pool(name="persist", bufs=1) as persist,
            tc.tile_pool(name="xs", bufs=8) as xs,
            tc.tile_pool(name="tmp", bufs=3) as tmp,
            tc.tile_pool(name="qk", bufs=2) as qk,
            tc.tile_pool(name="et", bufs=10) as et,
            tc.tile_pool(name="wb", bufs=4) as wb,
            tc.tile_pool(name="dram", bufs=1, space="DRAM") as dram,
        ):
            # ---- constants into SBUF ----
            # wq on the sync queue ahead of the xt stream; everything else on
            # the gpsimd queue.
            wq_sb = const.tile([P, MB, 256], bf16)
            nc.sync.dma_start(wq_sb[:], wq_h.rearrange("p (mb d) -> p mb d", mb=MB))
            wk_sb = const.tile([P, MB, HD], bf16)
            nc.gpsimd.dma_start(wk_sb[:], wk_h.rearrange("p (mb d) -> p mb d", mb=MB))
            wv_sb = const.tile([P, MB, HD], bf16)
            nc.gpsimd.dma_start(wv_sb[:], wv_h.rearrange("p (mb d) -> p mb d", mb=MB))
            cos_sb = const.tile([64, NS], bf16)
            nc.gpsimd.dma_start(cos_sb[:], cos_h)
            sin_sb = const.tile([64, NS], bf16)
            nc.gpsimd.dma_start(sin_sb[:], sin_h)
            mskb_sb = const.tile([P, 4 * 512], bf16)
            nc.gpsimd.dma_start(mskb_sb[:], mskb_h)
            wo_sb = const.tile([P, 2, MB, P], bf16)
            nc.gpsimd.dma_start(
                wo_sb[:], wo_h.rearrange("p (h mb d) -> p h mb d", h=2, mb=MB))
            ones_sb = const.tile([P, 1], bf16)
            nc.gpsimd.memset(ones_sb[:], 1.0)
            ones_row = const.tile([1, P], bf16)
            nc.gpsimd.memset(ones_row[:], 1.0)
            ident = const.tile([P, P], bf16)
            make_identity(nc, ident[:])

            # ---- per-batch persistent activations ----
            qrot = [persist.tile([P, 2, S], bf16, name=f"qrot{b}") for b in range(B)]
            krot = [persist.tile([P, S], bf16, name=f"krot{b}") for b in range(B)]
            vTt = [persist.tile([P, S], bf16, name=f"vTt{b}") for b in range(B)]
            vnat = [persist.tile([P, S // P, HD], bf16, name=f"vnat{b}")
                    for b in range(B)]
            oav = [persist.tile([P, 2, S], bf16, name=f"oav{b}") for b in range(B)]
            rs_in = [[dram.tile([NC * 256, 512], bf16, name=f"rsi{b}{t}")
                      for t in range(4)] for b in range(B)]
            rs_out = [[dram.tile([256, 512], bf16, name=f"rso{b}{t}")
                       for t in range(4)] for b in range(B)]

            def rope_unit(xe, xo, cos_c, sin_c, out_even, out_odd):
                # xe/xo are bf16 base-0 SBUF copies of the window's two halves
                t1 = tmp.tile([64, 512], bf16, tag="r1", name="r1")
                t2 = tmp.tile([64, 512], bf16, tag="r2", name="r2")
                nc.vector.tensor_mul(t1[:], xe, cos_c)
                nc.vector.tensor_mul(t2[:], xo, sin_c)
                nc.vector.tensor_sub(out_even, t1[:], t2[:])
                t3 = tmp.tile([64, 512], bf16, tag="r1", name="r3")
                t4 = tmp.tile([64, 512], bf16, tag="r2", name="r4")
                nc.vector.tensor_mul(t3[:], xe, sin_c)
                nc.vector.tensor_mul(t4[:], xo, cos_c)
                nc.vector.tensor_add(out_odd, t3[:], t4[:])

            def wo_partial(b, t):
                """Local partial output projection for chunk (b,t): 16 row
                blocks of wo_rows^T @ oav, cast bf16, stored to the RS input
                buffer on the DVE DMA queue, then ReduceScatter into out."""
                il = slice(t * 512, (t + 1) * 512)
                for ofb in range(MB):
                    pw = psS_pool[b].tile([P, 512], f32, tag="ps", name="pw")
                    for h in range(2):
                        nc.tensor.matmul(
                            pw[:], wo_sb[:, h, ofb, :], oav[b][:, h, il],
                            start=(h == 0), stop=(h == 1),
                        )
                    wt = wb.tile([P, 512], bf16, tag="w", name="wt")
                    if ofb % 2 == 0:
                        nc.scalar.copy(wt[:], pw[:])
                    else:
                        nc.vector.tensor_copy(wt[:], pw[:])
                    nc.scalar.dma_start(
                        rs_in[b][t][ofb * P:(ofb + 1) * P, :], wt[:])
                nc.gpsimd.collective_compute(
                    "ReduceScatter",
                    mybir.AluOpType.add,
                    replica_groups=[list(range(NC))],
                    ins=[rs_in[b][t].opt()],
                    outs=[rs_out[b][t].opt()],
                )
                nc.gpsimd.dma_start(out_h[b * 4 + t], rs_out[b][t][:])

            psS_pool = {}

            for b in range(B):
                # ---- projections (transposed layout), ACT eviction, RoPE ----
                with tc.tile_pool(name=f"psA{b}", bufs=2, space="PSUM") as psA:
                    for sp in range(4):          # 512-col windows within batch
                        gw = slice(b * S + sp * 512, b * S + (sp + 1) * 512)
                        lw = slice(sp * 512, (sp + 1) * 512)
                        pq = [psA.tile([P, 512], f32, tag=f"pq{h}", name=f"pq{h}")
                              for h in range(2)]
                        pk = psA.tile([P, 512], f32, tag="pk", name="pk")
                        pv = psA.tile([P, 512], f32, tag="pv", name="pv")
                        for m in range(MB):
                            xt = xs.tile([P, 512], bf16, tag="xt", name="xt")
                            nc.sync.dma_start(xt[:], xT_h[m, b * 4 + sp])
                            for acc, lhsT in (
                                (pq[0], wq_sb[:, m, 0:128]),
                                (pq[1], wq_sb[:, m, 128:256]),
                                (pk, wk_sb[:, m, :]),
                                (pv, wv_sb[:, m, :]),
                            ):
                                nc.tensor.matmul(
                                    acc[:], lhsT, xt[:],
                                    start=(m == 0), stop=(m == MB - 1),
                                )
                        # evict the PSUM banks to bf16 SBUF right away: even
                        # halves via ACT, odd halves via the DVE PSUM read
                        # port (which supports the base-64 partition offset)
                        halves = []
                        for u, src in enumerate((pq[0], pq[1], pk)):
                            xe = qk.tile([64, 512], bf16, tag=f"xe{u}",
                                         name=f"xe{u}")
                            xo = qk.tile([64, 512], bf16, tag=f"xo{u}",
                                         name=f"xo{u}")
                            nc.scalar.copy(xe[:], src[0:64, :])
                            nc.vector.tensor_copy(xo[:], src[64:128, :])
                            halves.append((xe, xo))
                        nc.scalar.copy(vTt[b][:, lw], pv[:])
                        # RoPE runs SBUF->SBUF on the DVE, off the PE path
                        cos_c, sin_c = cos_sb[:, gw], sin_sb[:, gw]
                        for h in range(2):
                            rope_unit(halves[h][0][:], halves[h][1][:],
                                      cos_c, sin_c,
                                      qrot[b][0:64, h, lw], qrot[b][64:128, h, lw])
                        rope_unit(halves[2][0][:], halves[2][1][:],
                                  cos_c, sin_c,
                                  krot[b][0:64, lw], krot[b][64:128, lw])

                # ---- v natural layout via PE transposes ----
                with tc.tile_pool(name=f"psT{b}", bufs=2, space="PSUM") as psT:
                    for blk in range(S // P):
                        pt = psT.tile([P, P], bf16, tag="pt", name="pt")
                        nc.tensor.transpose(
                            pt[:], vTt[b][:, blk * P:(blk + 1) * P], ident[:])
                        nc.scalar.copy(vnat[b][:, blk, :], pt[:])

                # ---- causal attention in scoresT layout ----
                with (
                    tc.tile_pool(name=f"psS{b}", bufs=4, space="PSUM") as psS,
                    tc.tile_pool(name=f"psV{b}", bufs=1, space="PSUM") as psV,
                    tc.tile_pool(name=f"psD{b}", bufs=1, space="PSUM") as psD,
                ):
                    psS_pool[b] = psS
                    for t in range(4):            # query chunks of 512
                        il = slice(t * 512, (t + 1) * 512)
                        pav = [psV.tile([P, 512], f32, tag=f"pav{h}",
                                        name=f"pav{h}") for h in range(2)]
                        pden = [psD.tile([1, 512], f32, tag=f"pden{h}",
                                         name=f"pden{h}") for h in range(2)]
                        nj = 4 * t + 4
                        # descending j: masked diagonal blocks run first so the
                        # drain at the end only waits on plain exps
                        order = list(range(nj - 1, -1, -1))

                        def av_den(e, idx, j, h):
                            nc.tensor.matmul(
                                pden[h][:], ones_sb[:], e[:],
                                start=(idx == 0), stop=(idx == nj - 1),
                            )
                            nc.tensor.matmul(
                                pav[h][:], vnat[b][:, j, :], e[:],
                                start=(idx == 0), stop=(idx == nj - 1),
                            )

                        pipe = []
                        for idx, j in enumerate(order):
                            for h in range(2):
                                rel = j - 4 * t
                                ps = psS.tile([P, 512], f32, tag="ps", name="ps")
                                nc.tensor.matmul(
                                    ps[:], krot[b][:, j * P:(j + 1) * P],
                                    qrot[b][:, h, il], start=True, stop=(rel < 0),
                                )
                                if rel >= 0:
                                    # causal mask as a -30 additive bias,
                                    # accumulated by the PE (identity matmul)
                                    nc.tensor.matmul(
                                        ps[:], ident[:],
                                        mskb_sb[:, rel * 512:(rel + 1) * 512],
                                        start=False, stop=True,
                                    )
                                e = et.tile([P, 512], bf16, tag="e", name="e")
                                nc.scalar.activation(e[:], ps[:], Exp)
                                pipe.append((e, idx, j, h))
                            while len(pipe) > 4:
                                av_den(*pipe.pop(0))
                        for item in pipe:
                            av_den(*item)

                        # ---- finalize: rcp(den) broadcast, one DVE mul/head
                        for h in range(2):
                            rcp = tmp.tile([1, 512], f32, tag="rcp", name="rcp")
                            nc.vector.reciprocal_approx_fast(rcp[:], pden[h][:])
                            rcp_bf = tmp.tile([1, 512], bf16, tag="rcpc",
                                              name="rcpc")
                            nc.vector.tensor_copy(rcp_bf[:], rcp[:])
                            rcp_ps = psS.tile([P, 512], f32, tag="ps", name="rb")
                            nc.tensor.matmul(rcp_ps[:], ones_row[:], rcp_bf[:],
                                             start=True, stop=True)
                            rcp_b = tmp.tile([P, 512], bf16, tag="rcpb",
                                             name="rcpb")
                            nc.scalar.copy(rcp_b[:], rcp_ps[:])
                            nc.vector.tensor_mul(oav[b][:, h, il],
                                                 pav[h][:], rcp_b[:])

                        # previous chunk's output projection fills the gap
                        # while this chunk's finalize chain completes
                        if t > 0:
                            wo_partial(b, t - 1)
                    wo_partial(b, 3)

    nc.compile()
    return nc


def _prep_inputs(x, freqs_cos, freqs_sin, wq, wk, wv, wo):
    x = np.asarray(x, np.float32).reshape(NS, DIM)
    xT = np.ascontiguousarray(
        x.T.reshape(MB, P, 8, 512).transpose(0, 2, 1, 3)).astype(BF)
    cos = np.asarray(freqs_cos, np.float32)
    sin = np.asarray(freqs_sin, np.float32)
    cosT = np.ascontiguousarray(np.tile(cos, (B, 1)).T).astype(BF)
    sinT = np.ascontiguousarray(np.tile(sin, (B, 1)).T).astype(BF)

    perm = np.r_[np.arange(0, HD, 2), np.arange(1, HD, 2)]
    scale = np.float32(1.0 / np.sqrt(HD))
    wq = np.asarray(wq, np.float32) * scale
    wk = np.asarray(wk, np.float32)
    wv = np.asarray(wv, np.float32)
    wo = np.asarray(wo, np.float32)

    masks = np.zeros((P, 4, 512), np.float32)
    for p in range(4):
        for isub in range(4):
            sl = slice(isub * 128, (isub + 1) * 128)
            if p < isub:
                masks[:, p, sl] = 1.0
            elif p == isub:
                masks[:, p, sl] = np.triu(np.ones((P, P), np.float32))
    maskb = np.ascontiguousarray(
        (-30.0 * (1.0 - masks)).reshape(P, 4 * 512)).astype(BF)

    def tile_w(w):
        # (2048, d) -> (128, 16*d): row mi holds [mb, d] contiguously
        d = w.shape[1]
        return np.ascontiguousarray(
            w.reshape(MB, P, d).transpose(1, 0, 2).reshape(P, MB * d)).astype(BF)

    in_maps = []
    for c in range(NC):
        wq_c = wq[:, c * 256:(c + 1) * 256]
        wq_cp = np.concatenate([wq_c[:, h * HD + perm] for h in range(2)], axis=1)
        # wo row slice for this core: rows [256c, 256(c+1)) over all 2048 cols,
        # laid out [d=128, h=2, ofb=16, of=128]
        wo_r = wo[c * 256:(c + 1) * 256, :].reshape(2, P, MB, P)
        wo_c = np.ascontiguousarray(
            wo_r.transpose(1, 0, 2, 3).reshape(P, 2 * MB * P)).astype(BF)
        in_maps.append({
            "xT": xT,
            "wq_c": tile_w(wq_cp),
            "wk_c": tile_w(wk[:, c * HD:(c + 1) * HD][:, perm]),
            "wv_c": tile_w(wv[:, c * HD:(c + 1) * HD]),
            "wo_c": wo_c,
            "cosT": cosT,
            "sinT": sinT,
            "maskb": maskb,
        })
    return in_maps


def _run(inputs, trace=False, **kw):
    from concourse.bass_utils import run_bass_kernel_spmd

    if "nc" not in _cache:
        _cache["nc"] = _build()
    nc = _cache["nc"]
    in_maps = _prep_inputs(**inputs)
    res = run_bass_kernel_spmd(
        nc, in_maps, core_ids=list(range(NC)), trace=trace, **kw
    )
    out = np.empty((NS, DIM), np.float32)
    for c in range(NC):
        chunks = np.asarray(res.results[c]["outT"], dtype=np.float32)
        for b in range(B):
            for t in range(4):
                out[b * S + t * 512:b * S + (t + 1) * 512,
                    c * 256:(c + 1) * 256] = chunks[b * 4 + t].T
    return out.reshape(B, S, DIM), res


def kernel(**inputs) -> np.ndarray:
    out, _ = _run(inputs, trace=False)
    return out


# revision 9
# speedup vs baseline: 1.0172x; 1.0172x over previous
"""Trainium2 Bass kernel for GQA attention (B=2, S=2048, DIM=2048, H=16, KV=8,
HD=128) with RoPE + causal mask + output projection.

Sharding: 8-way tensor parallelism over heads. Core c computes q heads
{2c, 2c+1} and kv head c end-to-end (QKV projection, RoPE, causal attention),
then multiplies its own attention output by its ROW slice of wo (rows
256c:256(c+1)), producing a partial full-width output; a ReduceScatter sums
partials across cores and hands core c the output-feature rows
[256c:256(c+1)], which the host transposes/concats. The PE never waits on a
collective: all matmul work is local.

v4 pipeline notes (vs v3 baseline):
- softmax denominator accumulated on the PE (ones-vector matmul per j block,
  PSUM-accumulated alongside the AV matmul) instead of a DVE add chain; kills
  ~110us of DVE work and the per-chunk PE stalls waiting on it.
- projection PSUM windows are evicted to SBUF bf16 by the ACT engine
  immediately; RoPE runs SBUF->SBUF on the DVE in bf16 off the critical path
  (PSUM banks free ~2us after each window instead of ~12us).
- output projection computes per-core partials from local SBUF activations
  (no AllGather, no gather loads); partial tiles are cast/stored per
  128-row block and ReduceScattered straight into the output tensor.
- wo partial stores ride the DVE DMA queue; xt stream keeps the sync queue;
  weights/collectives ride the gpsimd queue. wq alone loads first on sync.
- finalize: reciprocal_approx_fast on [1,512] denominators, broadcast via
  ones-row matmul, ACT copy to bf16, single DVE multiply per head.

Layout tricks (unchanged from v3):
- everything computed transposed (feature dim on SBUF partitions); only
  on-device transposes are 16 PE transposes per batch for v.
- RoPE interleaved pairs handled by permuting wq/wk columns on the host to
  [evens, odds] per head; q/k permuted consistently so dot products are
  unchanged; v / wo stay unpermuted.
- softmax in scoresT layout (keys on partitions): no max subtraction (scores
  are O(5)), causal mask as a -30 additive bias accumulated by the PE
  (identity matmul) on diagonal blocks only.
- matmuls bf16 (fp32 accumulate); 1/sqrt(HD) folded into wq.
"""

import sys

if "/opt/trn_rl_repo" not in sys.path:
    sys.path.insert(0, "/opt/trn_rl_repo")

import numpy as np
import ml_dtypes

B, S, DIM = 2, 2048, 2048
H, KV, HD = 16, 8, 128
NC = 8
NS = B * S            # 4096 flattened (b, s) rows
P = 128
MB = DIM // P         # 16 contraction blocks for the projections
BF = ml_dtypes.bfloat16

_cache: dict = {}


def _build(debug=False):
    import concourse.bass as bass
    import concourse.mybir as mybir
    import concourse.tile as tile
    from concourse import bacc
    from concourse.masks import make_identity

    dt = mybir.dt
    f32, bf16 = dt.float32, dt.bfloat16
    Exp = mybir.ActivationFunctionType.Exp

    nc = bacc.Bacc("TRN2", debug=False, target_bir_lowering=False, num_devices=NC)

    # x^T arrives pre-tiled as [m_block, window, 128, 512] so every
    # projection-stream DMA is one contiguous 128KB block
    xT_h = nc.dram_tensor("xT", (MB, 8, P, 512), bf16, kind="ExternalInput").ap()
    # weights arrive pre-tiled as [mi=128, mb*d] so their DMAs are contiguous
    wq_h = nc.dram_tensor("wq_c", (P, MB * 256), bf16, kind="ExternalInput").ap()
    wk_h = nc.dram_tensor("wk_c", (P, MB * HD), bf16, kind="ExternalInput").ap()
    wv_h = nc.dram_tensor("wv_c", (P, MB * HD), bf16, kind="ExternalInput").ap()
    # wo row-slice for this core: [d=128, h=2, ofb=16, of=128]
    wo_h = nc.dram_tensor("wo_c", (P, 2 * MB * P), bf16, kind="ExternalInput").ap()
    cos_h = nc.dram_tensor("cosT", (64, NS), bf16, kind="ExternalInput").ap()
    sin_h = nc.dram_tensor("sinT", (64, NS), bf16, kind="ExternalInput").ap()
    mskb_h = nc.dram_tensor("maskb", (P, 4 * 512), bf16, kind="ExternalInput").ap()
    # per-(b,t) chunk of this core's 256 output-feature rows, from ReduceScatter
    out_h = nc.dram_tensor("outT", (B * 4, 256, 512), bf16,
                           kind="ExternalOutput").ap()

    with tile.TileContext(nc) as tc:
        with (
            tc.tile_pool(name="const", bufs=1) as const,
            tc.tile_pool(name="persist", bufs=1) as persist,
            tc.tile_pool(name="xs", bufs=8) as xs,
            tc.tile_pool(name="tmp", bufs=3) as tmp,
            tc.tile_pool(name="qk", bufs=2) as qk,
            tc.tile_pool(name="et", bufs=10) as et,
            tc.tile_pool(name="wb", bufs=4) as wb,
            tc.tile_pool(name="dram", bufs=1, space="DRAM") as dram,
        ):
            # ---- constants into SBUF ----
            # wq on the sync queue ahead of the xt stream; everything else on
            # the gpsimd queue.
            wq_sb = const.tile([P, MB, 256], bf16)
            nc.sync.dma_start(wq_sb[:], wq_h.rearrange("p (mb d) -> p mb d", mb=MB))
            wk_sb = const.tile([P, MB, HD], bf16)
            nc.gpsimd.dma_start(wk_sb[:], wk_h.rearrange("p (mb d) -> p mb d", mb=MB))
            wv_sb = const.tile([P, MB, HD], bf16)
            nc.gpsimd.dma_start(wv_sb[:], wv_h.rearrange("p (mb d) -> p mb d", mb=MB))
            cos_sb = const.tile([64, NS], bf16)
            nc.gpsimd.dma_start(cos_sb[:], cos_h)
            sin_sb = const.tile([64, NS], bf16)
            nc.gpsimd.dma_start(sin_sb[:], sin_h)
            mskb_sb = const.tile([P, 4 * 512], bf16)
            nc.gpsimd.dma_start(mskb_sb[:], mskb_h)
            wo_sb = const.tile([P, 2, MB, P], bf16)
            nc.gpsimd.dma_start(
                wo_sb[:], wo_h.rearrange("p (h mb d) -> p h mb d", h=2, mb=MB))
            ones_sb = const.tile([P, 1], bf16)
            nc.gpsimd.memset(ones_sb[:], 1.0)
            ones_row = const.tile([1, P], bf16)
            nc.gpsimd.memset(ones_row[:], 1.0)
            ident = const.tile([P, P], bf16)
            make_identity(nc, ident[:])

            # ---- per-batch persistent activations ----
            qrot = [persist.tile([P, 2, S], bf16, name=f"qrot{b}") for b in range(B)]
            krot = [persist.tile([P, S], bf16, name=f"krot{b}") for b in range(B)]
            vTt = [persist.tile([P, S], bf16, name=f"vTt{b}") for b in range(B)]
            vnat = [persist.tile([P, S // P, HD], bf16, name=f"vnat{b}")
                    for b in range(B)]
            oav = [persist.tile([P, 2, S], bf16, name=f"oav{b}") for b in range(B)]
            rs_in = [[dram.tile([NC * 256, 512], bf16, name=f"rsi{b}{t}")
                      for t in range(4)] for b in range(B)]
            rs_out = [[dram.tile([256, 512], bf16, name=f"rso{b}{t}")
                       for t in range(4)] for b in range(B)]

            def rope_unit(xe, xo, cos_c, sin_c, out_even, out_odd):
                # xe/xo are bf16 base-0 SBUF copies of the window's two halves
                t1 = tmp.tile([64, 512], bf16, tag="r1", name="r1")
                t2 = tmp.tile([64, 512], bf16, tag="r2", name="r2")
                nc.vector.tensor_mul(t1[:], xe, cos_c)
                nc.vector.tensor_mul(t2[:], xo, sin_c)
                nc.vector.tensor_sub(out_even, t1[:], t2[:])
                t3 = tmp.tile([64, 512], bf16, tag="r1", name="r3")
                t4 = tmp.tile([64, 512], bf16, tag="r2", name="r4")
                nc.vector.tensor_mul(t3[:], xe, sin_c)
                nc.vector.tensor_mul(t4[:], xo, cos_c)
                nc.vector.tensor_add(out_odd, t3[:], t4[:])

            def wo_partial(b, t):
                """Local partial output projection for chunk (b,t): 16 row
                blocks of wo_rows^T @ oav, cast bf16, stored to the RS input
                buffer on the DVE DMA queue, then ReduceScatter into out."""
                il = slice(t * 512, (t + 1) * 512)
                for ofb in range(MB):
                    pw = psS_pool[b].tile([P, 512], f32, tag="ps", name="pw")
                    for h in range(2):
                        nc.tensor.matmul(
                            pw[:], wo_sb[:, h, ofb, :], oav[b][:, h, il],
                            start=(h == 0), stop=(h == 1),
                        )
                    wt = wb.tile([P, 512], bf16, tag="w", name="wt")
                    nc.vector.tensor_copy(wt[:], pw[:])
                    nc.sync.dma_start(
                        rs_in[b][t][ofb * P:(ofb + 1) * P, :], wt[:])
                nc.gpsimd.collective_compute(
                    "ReduceScatter",
                    mybir.AluOpType.add,
                    replica_groups=[list(range(NC))],
                    ins=[rs_in[b][t].opt()],
                    outs=[rs_out[b][t].opt()],
                )
                nc.gpsimd.dma_start(out_h[b * 4 + t], rs_out[b][t][:])

            psS_pool = {}

            for b in range(B):
                # ---- projections (transposed layout), ACT eviction, RoPE ----
                with tc.tile_pool(name=f"psA{b}", bufs=2, space="PSUM") as psA:
                    for sp in range(4):          # 512-col windows within batch
                        gw = slice(b * S + sp * 512, b * S + (sp + 1) * 512)
                        lw = slice(sp * 512, (sp + 1) * 512)
                        pq = [psA.tile([P, 512], f32, tag=f"pq{h}", name=f"pq{h}")
                              for h in range(2)]
                        pk = psA.tile([P, 512], f32, tag="pk", name="pk")
                        pv = psA.tile([P, 512], f32, tag="pv", name="pv")
                        for m in range(MB):
                            xt = xs.tile([P, 512], bf16, tag="xt", name="xt")
                            nc.sync.dma_start(xt[:], xT_h[m, b * 4 + sp])
                            for acc, lhsT in (
                                (pq[0], wq_sb[:, m, 0:128]),
                                (pq[1], wq_sb[:, m, 128:256]),
                                (pk, wk_sb[:, m, :]),
                                (pv, wv_sb[:, m, :]),
                            ):
                                nc.tensor.matmul(
                                    acc[:], lhsT, xt[:],
                                    start=(m == 0), stop=(m == MB - 1),
                                )
                        # evict the PSUM banks to bf16 SBUF right away: even
                        # halves via ACT, odd halves via the DVE PSUM read
                        # port (which supports the base-64 partition offset)
                        halves = []
                        for u, src in enumerate((pq[0], pq[1], pk)):
                            xe = qk.tile([64, 512], bf16, tag=f"xe{u}",
                                         name=f"xe{u}")
                            xo = qk.tile([64, 512], bf16, tag=f"xo{u}",
                                         name=f"xo{u}")
                            nc.scalar.copy(xe[:], src[0:64, :])
                            nc.vector.tensor_copy(xo[:], src[64:128, :])
                            halves.append((xe, xo))
                        nc.scalar.copy(vTt[b][:, lw], pv[:])
                        # RoPE runs SBUF->SBUF on the DVE, off the PE path
                        cos_c, sin_c = cos_sb[:, gw], sin_sb[:, gw]
                        for h in range(2):
                            rope_unit(halves[h][0][:], halves[h][1][:],
                                      cos_c, sin_c,
                                      qrot[b][0:64, h, lw], qrot[b][64:128, h, lw])
                        rope_unit(halves[2][0][:], halves[2][1][:],
                                  cos_c, sin_c,
                                  krot[b][0:64, lw], krot[b][64:128, lw])

                # ---- v natural layout via PE transposes ----
                with tc.tile_pool(name=f"psT{b}", bufs=2, space="PSUM") as psT:
                    for blk in range(S // P):
                        pt = psT.tile([P, P], bf16, tag="pt", name="pt")
                        nc.tensor.transpose(
                            pt[:], vTt[b][:, blk * P:(blk + 1) * P], ident[:])
                        nc.scalar.copy(vnat[b][:, blk, :], pt[:])

                # ---- causal attention in scoresT layout ----
                with (
                    tc.tile_pool(name=f"psS{b}", bufs=4, space="PSUM") as psS,
                    tc.tile_pool(name=f"psV{b}", bufs=1, space="PSUM") as psV,
                    tc.tile_pool(name=f"psD{b}", bufs=1, space="PSUM") as psD,
                ):
                    psS_pool[b] = psS
                    for t in range(4):            # query chunks of 512
                        il = slice(t * 512, (t + 1) * 512)
                        pav = [psV.tile([P, 512], f32, tag=f"pav{h}",
                                        name=f"pav{h}") for h in range(2)]
                        pden = [psD.tile([1, 512], f32, tag=f"pden{h}",
                                         name=f"pden{h}") for h in range(2)]
                        nj = 4 * t + 4
                        # descending j: masked diagonal blocks run first so the
                        # drain at the end only waits on plain exps
                        order = list(range(nj - 1, -1, -1))

                        def av_den(e, idx, j, h):
                            nc.tensor.matmul(
                                pden[h][:], ones_sb[:], e[:],
                                start=(idx == 0), stop=(idx == nj - 1),
                            )
                            nc.tensor.matmul(
                                pav[h][:], vnat[b][:, j, :], e[:],
                                start=(idx == 0), stop=(idx == nj - 1),
                            )

                        pipe = []
                        for idx, j in enumerate(order):
                            for h in range(2):
                                rel = j - 4 * t
                                ps = psS.tile([P, 512], f32, tag="ps", name="ps")
                                nc.tensor.matmul(
                                    ps[:], krot[b][:, j * P:(j + 1) * P],
                                    qrot[b][:, h, il], start=True, stop=(rel < 0),
                                )
                                if rel >= 0:
                                    # causal mask as a -30 additive bias,
                                    # accumulated by the PE (identity matmul)
                                    nc.tensor.matmul(
                                        ps[:], ident[:],
                                        mskb_sb[:, rel * 512:(rel + 1) * 512],
                                        start=False, stop=True,
                                    )
                                e = et.tile([P, 512], bf16, tag="e", name="e")
                                nc.scalar.activation(e[:], ps[:], Exp)
                                pipe.append((e, idx, j, h))
                            while len(pipe) > 4:
                                av_den(*pipe.pop(0))
                        for item in pipe:
                            av_den(*item)

                        # ---- finalize: rcp(den) broadcast, one DVE mul/head
                        for h in range(2):
                            rcp = tmp.tile([1, 512], f32, tag="rcp", name="rcp")
                            nc.vector.reciprocal_approx_fast(rcp[:], pden[h][:])
                            rcp_bf = tmp.tile([1, 512], bf16, tag="rcpc",
                                              name="rcpc")
                            nc.vector.tensor_copy(rcp_bf[:], rcp[:])
                            rcp_ps = psS.tile([P, 512], f32, tag="ps", name="rb")
                            nc.tensor.matmul(rcp_ps[:], ones_row[:], rcp_bf[:],
                                             start=True, stop=True)
                            rcp_b = tmp.tile([P, 512], bf16, tag="rcpb",
                                             name="rcpb")
                            nc.scalar.copy(rcp_b[:], rcp_ps[:])
                            nc.vector.tensor_mul(oav[b][:, h, il],
                                                 pav[h][:], rcp_b[:])

                        # previous chunk's output projection fills the gap
                        # while this chunk's finalize chain completes
                        if t > 0:
                            wo_partial(b, t - 1)
                    wo_partial(b, 3)

    nc.compile()
    return nc


def _prep_inputs(x, freqs_cos, freqs_sin, wq, wk, wv, wo):
    x = np.asarray(x, np.float32).reshape(NS, DIM)
    xT = np.ascontiguousarray(
        x.T.reshape(MB, P, 8, 512).transpose(0, 2, 1, 3)).astype(BF)
    cos = np.asarray(freqs_cos, np.float32)
    sin = np.asarray(freqs_sin, np.float32)
    cosT = np.ascontiguousarray(np.tile(cos, (B, 1)).T).astype(BF)
    sinT = np.ascontiguousarray(np.tile(sin, (B, 1)).T).astype(BF)

    perm = np.r_[np.arange(0, HD, 2), np.arange(1, HD, 2)]
    scale = np.float32(1.0 / np.sqrt(HD))
    wq = np.asarray(wq, np.float32) * scale
    wk = np.asarray(wk, np.float32)
    wv = np.asarray(wv, np.float32)
    wo = np.asarray(wo, np.float32)

    masks = np.zeros((P, 4, 512), np.float32)
    for p in range(4):
        for isub in range(4):
            sl = slice(isub * 128, (isub + 1) * 128)
            if p < isub:
                masks[:, p, sl] = 1.0
            elif p == isub:
                masks[:, p, sl] = np.triu(np.ones((P, P), np.float32))
    maskb = np.ascontiguousarray(
        (-30.0 * (1.0 - masks)).reshape(P, 4 * 512)).astype(BF)

    def tile_w(w):
        # (2048, d) -> (128, 16*d): row mi holds [mb, d] contiguously
        d = w.shape[1]
        return np.ascontiguousarray(
            w.reshape(MB, P, d).transpose(1, 0, 2).reshape(P, MB * d)).astype(BF)

    in_maps = []
    for c in range(NC):
        wq_c = wq[:, c * 256:(c + 1) * 256]
        wq_cp = np.concatenate([wq_c[:, h * HD + perm] for h in range(2)], axis=1)
        # wo row slice for this core: rows [256c, 256(c+1)) over all 2048 cols,
        # laid out [d=128, h=2, ofb=16, of=128]
        wo_r = wo[c * 256:(c + 1) * 256, :].reshape(2, P, MB, P)
        wo_c = np.ascontiguousarray(
            wo_r.transpose(1, 0, 2, 3).reshape(P, 2 * MB * P)).astype(BF)
        in_maps.append({
            "xT": xT,
            "wq_c": tile_w(wq_cp),
            "wk_c": tile_w(wk[:, c * HD:(c + 1) * HD][:, perm]),
            "wv_c": tile_w(wv[:, c * HD:(c + 1) * HD]),
            "wo_c": wo_c,
            "cosT": cosT,
            "sinT": sinT,
            "maskb": maskb,
        })
    return in_maps


def _run(inputs, trace=False, **kw):
    from concourse.bass_utils import run_bass_kernel_spmd

    if "nc" not in _cache:
        _cache["nc"] = _build()
    nc = _cache["nc"]
    in_maps = _prep_inputs(**inputs)
    res = run_bass_kernel_spmd(
        nc, in_maps, core_ids=list(range(NC)), trace=trace, **kw
    )
    out = np.empty((NS, DIM), np.float32)
    for c in range(NC):
        chunks = np.asarray(res.results[c]["outT"], dtype=np.float32)
        for b in range(B):
            for t in range(4):
                out[b * S + t * 512:b * S + (t + 1) * 512,
                    c * 256:(c + 1) * 256] = chunks[b * 4 + t].T
    return out.reshape(B, S, DIM), res


def kernel(**inputs) -> np.ndarray:
    out, _ = _run(inputs, trace=False)
    return out


# revision 10
# speedup vs baseline: 1.2053x; 1.1849x over previous
"""Trainium2 Bass kernel for GQA attention (B=2, S=2048, DIM=2048, H=16, KV=8,
HD=128) with RoPE + causal mask + output projection.

Sharding: 8-way tensor parallelism over heads. Core c computes q heads
{2c, 2c+1} and kv head c end-to-end (QKV projection, RoPE, causal attention),
contributes its transposed attention output to on-device AllGathers (Shared
output buffers), then computes the output-projection column slice
out[:, 256c:256(c+1)] from the gathered activations. The host only slices
inputs and concatenates outputs.

v5 pipeline notes:
- softmax denominator accumulated on the PE (ones-vector matmul per j block,
  PSUM-accumulated alongside the AV matmul) instead of a DVE add chain.
- projection PSUM windows evicted to bf16 SBUF immediately (ACT for the even
  halves, DVE PSUM-read-port copies for the odd halves); RoPE runs
  SBUF->SBUF in bf16 on the DVE off the PE critical path.
- finalize per chunk: reciprocal_approx_fast on the [1,512] denominator,
  ones-row broadcast matmul, ACT copy to bf16, one DVE multiply per head.
- wo block for chunk (b,t) is emitted two attention chunks after its
  AllGather fires, so the PE arrives after the collective completes; gather
  loads ride the gpsimd queue so they can never head-of-line-block the xt
  stream on the sync queue.
- AllGather outputs are Shared-address-space DRAM (faster RDH path).

Layout tricks:
- everything computed transposed (feature dim on SBUF partitions); only
  on-device transposes are 16 PE transposes per batch for v.
- RoPE interleaved pairs handled by permuting wq/wk columns on the host to
  [evens, odds] per head; q/k permuted consistently so dot products are
  unchanged; v / wo stay unpermuted.
- softmax in scoresT layout (keys on partitions): no max subtraction (scores
  are O(5)), causal mask as a -30 additive bias accumulated by the PE
  (identity matmul) on diagonal blocks only.
- matmuls bf16 (fp32 accumulate); 1/sqrt(HD) folded into wq.
"""

import sys

if "/opt/trn_rl_repo" not in sys.path:
    sys.path.insert(0, "/opt/trn_rl_repo")

import numpy as np
import ml_dtypes

B, S, DIM = 2, 2048, 2048
H, KV, HD = 16, 8, 128
NC = 8
NS = B * S            # 4096 flattened (b, s) rows
P = 128
MB = DIM // P         # 16 contraction blocks for the projections
BF = ml_dtypes.bfloat16

_cache: dict = {}


def _build(debug=False):
    import concourse.bass as bass
    import concourse.mybir as mybir
    import concourse.tile as tile
    from concourse import bacc
    from concourse.masks import make_identity

    dt = mybir.dt
    f32, bf16 = dt.float32, dt.bfloat16
    Exp = mybir.ActivationFunctionType.Exp

    nc = bacc.Bacc("TRN2", debug=False, target_bir_lowering=False, num_devices=NC)

    # x^T arrives pre-tiled as [m_block, window, 128, 512] so every
    # projection-stream DMA is one contiguous 128KB block
    xT_h = nc.dram_tensor("xT", (MB, 8, P, 512), bf16, kind="ExternalInput").ap()
    # weights arrive pre-tiled as [mi=128, mb*d] so their DMAs are contiguous
    wq_h = nc.dram_tensor("wq_c", (P, MB * 256), bf16, kind="ExternalInput").ap()
    wk_h = nc.dram_tensor("wk_c", (P, MB * HD), bf16, kind="ExternalInput").ap()
    wv_h = nc.dram_tensor("wv_c", (P, MB * HD), bf16, kind="ExternalInput").ap()
    wo_h = nc.dram_tensor("wo_c", (P, MB * 256), bf16, kind="ExternalInput").ap()
    cos_h = nc.dram_tensor("cosT", (64, NS), bf16, kind="ExternalInput").ap()
    sin_h = nc.dram_tensor("sinT", (64, NS), bf16, kind="ExternalInput").ap()
    mskb_h = nc.dram_tensor("maskb", (P, 4 * 512), bf16, kind="ExternalInput").ap()
    out_h = nc.dram_tensor("outT", (256, NS), f32, kind="ExternalOutput").ap()

    with tile.TileContext(nc) as tc:
        with (
            tc.tile_pool(name="const", bufs=1) as const,
            tc.tile_pool(name="persist", bufs=1) as persist,
            tc.tile_pool(name="xs", bufs=8) as xs,
            tc.tile_pool(name="tmp", bufs=3) as tmp,
            tc.tile_pool(name="qk", bufs=2) as qk,
            tc.tile_pool(name="et", bufs=10) as et,
            tc.tile_pool(name="gp", bufs=8) as gp,
            tc.tile_pool(name="ot", bufs=3) as ot,
            tc.tile_pool(name="dram", bufs=1, space="DRAM") as dram,
        ):
            # ---- constants into SBUF ----
            # wq on the sync queue ahead of the xt stream; everything else on
            # the gpsimd queue.
            wq_sb = const.tile([P, MB, 256], bf16)
            nc.sync.dma_start(wq_sb[:], wq_h.rearrange("p (mb d) -> p mb d", mb=MB))
            wk_sb = const.tile([P, MB, HD], bf16)
            nc.gpsimd.dma_start(wk_sb[:], wk_h.rearrange("p (mb d) -> p mb d", mb=MB))
            wv_sb = const.tile([P, MB, HD], bf16)
            nc.gpsimd.dma_start(wv_sb[:], wv_h.rearrange("p (mb d) -> p mb d", mb=MB))
            cos_sb = const.tile([64, NS], bf16)
            nc.gpsimd.dma_start(cos_sb[:], cos_h)
            sin_sb = const.tile([64, NS], bf16)
            nc.gpsimd.dma_start(sin_sb[:], sin_h)
            mskb_sb = const.tile([P, 4 * 512], bf16)
            nc.gpsimd.dma_start(mskb_sb[:], mskb_h)
            wo_sb = const.tile([P, MB, 256], bf16)
            nc.gpsimd.dma_start(wo_sb[:], wo_h.rearrange("p (mb d) -> p mb d", mb=MB))
            ones_sb = const.tile([P, 1], bf16)
            nc.gpsimd.memset(ones_sb[:], 1.0)
            ones_row = const.tile([1, P], bf16)
            nc.gpsimd.memset(ones_row[:], 1.0)
            ident = const.tile([P, P], bf16)
            make_identity(nc, ident[:])

            # ---- per-batch persistent activations ----
            qrot = [persist.tile([P, 2, S], bf16, name=f"qrot{b}") for b in range(B)]
            krot = [persist.tile([P, S], bf16, name=f"krot{b}") for b in range(B)]
            vTt = [persist.tile([P, S], bf16, name=f"vTt{b}") for b in range(B)]
            vnat = [persist.tile([P, S // P, HD], bf16, name=f"vnat{b}")
                    for b in range(B)]
            oav = [persist.tile([P, 2, S], bf16, name=f"oav{b}") for b in range(B)]
            ag_in = [[dram.tile([256, 512], bf16, name=f"agi{b}{t}")
                      for t in range(4)] for b in range(B)]
            ag_out = [[dram.tile([NC * 256, 512], bf16, name=f"ago{b}{t}",
                                 addr_space="Shared")
                       for t in range(4)] for b in range(B)]

            def rope_unit(xe, xo, cos_c, sin_c, out_even, out_odd):
                # xe/xo are bf16 base-0 SBUF copies of the window's two halves
                t1 = tmp.tile([64, 512], bf16, tag="r1", name="r1")
                t2 = tmp.tile([64, 512], bf16, tag="r2", name="r2")
                nc.vector.tensor_mul(t1[:], xe, cos_c)
                nc.vector.tensor_mul(t2[:], xo, sin_c)
                nc.vector.tensor_sub(out_even, t1[:], t2[:])
                t3 = tmp.tile([64, 512], bf16, tag="r1", name="r3")
                t4 = tmp.tile([64, 512], bf16, tag="r2", name="r4")
                nc.vector.tensor_mul(t3[:], xe, sin_c)
                nc.vector.tensor_mul(t4[:], xo, cos_c)
                nc.vector.tensor_add(out_odd, t3[:], t4[:])

            def wo_exchange(b, t):
                """Stage chunk (b,t)'s attention output and fire its
                AllGather (both on the gpsimd queue)."""
                il = slice(t * 512, (t + 1) * 512)
                for h in range(2):
                    nc.gpsimd.dma_start(
                        ag_in[b][t][h * P:(h + 1) * P, :], oav[b][:, h, il])
                nc.gpsimd.collective_compute(
                    "AllGather",
                    mybir.AluOpType.bypass,
                    replica_groups=[list(range(NC))],
                    ins=[ag_in[b][t].opt()],
                    outs=[ag_out[b][t].opt()],
                )

            def wo_block(b, t, pool):
                """Output-projection column slice for chunk (b,t): 16 gather
                loads (gpsimd queue) + 32 PE matmuls + f32 eviction/store."""
                pw = [pool.tile([P, 512], f32, tag="ps", name=f"pw{n}")
                      for n in range(2)]
                for r in range(MB):
                    g = gp.tile([P, 512], bf16, tag="g", name="g")
                    nc.gpsimd.dma_start(g[:], ag_out[b][t][r * P:(r + 1) * P, :])
                    for n in range(2):
                        nc.tensor.matmul(
                            pw[n][:], wo_sb[:, r, n * 128:(n + 1) * 128],
                            g[:], start=(r == 0), stop=(r == MB - 1),
                        )
                for n in range(2):
                    o = ot.tile([P, 512], f32, tag="o", name="o")
                    nc.scalar.copy(o[:], pw[n][:])
                    nc.sync.dma_start(
                        out_h[n * P:(n + 1) * P,
                              b * S + t * 512: b * S + (t + 1) * 512],
                        o[:],
                    )

            # wo blocks are deferred two chunks behind their AllGather; this
            # queue carries (b, t) pairs whose exchange has fired
            wo_pending = []

            for b in range(B):
                # ---- projections (transposed layout), eviction, RoPE ----
                with tc.tile_pool(name=f"psA{b}", bufs=2, space="PSUM") as psA:
                    for sp in range(4):          # 512-col windows within batch
                        gw = slice(b * S + sp * 512, b * S + (sp + 1) * 512)
                        lw = slice(sp * 512, (sp + 1) * 512)
                        pq = [psA.tile([P, 512], f32, tag=f"pq{h}", name=f"pq{h}")
                              for h in range(2)]
                        pk = psA.tile([P, 512], f32, tag="pk", name="pk")
                        pv = psA.tile([P, 512], f32, tag="pv", name="pv")
                        for m in range(MB):
                            xt = xs.tile([P, 512], bf16, tag="xt", name="xt")
                            nc.sync.dma_start(xt[:], xT_h[m, b * 4 + sp])
                            for acc, lhsT in (
                                (pq[0], wq_sb[:, m, 0:128]),
                                (pq[1], wq_sb[:, m, 128:256]),
                                (pk, wk_sb[:, m, :]),
                                (pv, wv_sb[:, m, :]),
                            ):
                                nc.tensor.matmul(
                                    acc[:], lhsT, xt[:],
                                    start=(m == 0), stop=(m == MB - 1),
                                )
                        # evict the PSUM banks to bf16 SBUF right away: even
                        # halves via ACT, odd halves via the DVE PSUM read
                        # port (which supports the base-64 partition offset)
                        halves = []
                        for u, src in enumerate((pq[0], pq[1], pk)):
                            xe = qk.tile([64, 512], bf16, tag=f"xe{u}",
                                         name=f"xe{u}")
                            xo = qk.tile([64, 512], bf16, tag=f"xo{u}",
                                         name=f"xo{u}")
                            nc.scalar.copy(xe[:], src[0:64, :])
                            nc.vector.tensor_copy(xo[:], src[64:128, :])
                            halves.append((xe, xo))
                        nc.scalar.copy(vTt[b][:, lw], pv[:])
                        # RoPE runs SBUF->SBUF on the DVE, off the PE path
                        cos_c, sin_c = cos_sb[:, gw], sin_sb[:, gw]
                        for h in range(2):
                            rope_unit(halves[h][0][:], halves[h][1][:],
                                      cos_c, sin_c,
                                      qrot[b][0:64, h, lw], qrot[b][64:128, h, lw])
                        rope_unit(halves[2][0][:], halves[2][1][:],
                                  cos_c, sin_c,
                                  krot[b][0:64, lw], krot[b][64:128, lw])

                # ---- v natural layout via PE transposes ----
                with tc.tile_pool(name=f"psT{b}", bufs=2, space="PSUM") as psT:
                    for blk in range(S // P):
                        pt = psT.tile([P, P], bf16, tag="pt", name="pt")
                        nc.tensor.transpose(
                            pt[:], vTt[b][:, blk * P:(blk + 1) * P], ident[:])
                        nc.scalar.copy(vnat[b][:, blk, :], pt[:])

                # ---- causal attention in scoresT layout ----
                with (
                    tc.tile_pool(name=f"psS{b}", bufs=4, space="PSUM") as psS,
                    tc.tile_pool(name=f"psV{b}", bufs=1, space="PSUM") as psV,
                    tc.tile_pool(name=f"psD{b}", bufs=1, space="PSUM") as psD,
                ):
                    for t in range(4):            # query chunks of 512
                        il = slice(t * 512, (t + 1) * 512)
                        pav = [psV.tile([P, 512], f32, tag=f"pav{h}",
                                        name=f"pav{h}") for h in range(2)]
                        pden = [psD.tile([1, 512], f32, tag=f"pden{h}",
                                         name=f"pden{h}") for h in range(2)]
                        nj = 4 * t + 4
                        # descending j: masked diagonal blocks run first so the
                        # drain at the end only waits on plain exps
                        order = list(range(nj - 1, -1, -1))

                        def av_den(e, idx, j, h):
                            nc.tensor.matmul(
                                pden[h][:], ones_sb[:], e[:],
                                start=(idx == 0), stop=(idx == nj - 1),
                            )
                            nc.tensor.matmul(
                                pav[h][:], vnat[b][:, j, :], e[:],
                                start=(idx == 0), stop=(idx == nj - 1),
                            )

                        pipe = []
                        for idx, j in enumerate(order):
                            for h in range(2):
                                rel = j - 4 * t
                                ps = psS.tile([P, 512], f32, tag="ps", name="ps")
                                nc.tensor.matmul(
                                    ps[:], krot[b][:, j * P:(j + 1) * P],
                                    qrot[b][:, h, il], start=True, stop=(rel < 0),
                                )
                                if rel >= 0:
                                    # causal mask as a -30 additive bias,
                                    # accumulated by the PE (identity matmul)
                                    nc.tensor.matmul(
                                        ps[:], ident[:],
                                        mskb_sb[:, rel * 512:(rel + 1) * 512],
                                        start=False, stop=True,
                                    )
                                e = et.tile([P, 512], bf16, tag="e", name="e")
                                nc.scalar.activation(e[:], ps[:], Exp)
                                pipe.append((e, idx, j, h))
                            while len(pipe) > 4:
                                av_den(*pipe.pop(0))
                        for item in pipe:
                            av_den(*item)

                        # ---- finalize: rcp(den) broadcast, one DVE mul/head
                        for h in range(2):
                            rcp = tmp.tile([1, 512], f32, tag="rcp", name="rcp")
                            nc.vector.reciprocal_approx_fast(rcp[:], pden[h][:])
                            rcp_bf = tmp.tile([1, 512], bf16, tag="rcpc",
                                              name="rcpc")
                            nc.vector.tensor_copy(rcp_bf[:], rcp[:])
                            rcp_ps = psS.tile([P, 512], f32, tag="ps", name="rb")
                            nc.tensor.matmul(rcp_ps[:], ones_row[:], rcp_bf[:],
                                             start=True, stop=True)
                            rcp_b = tmp.tile([P, 512], bf16, tag="rcpb",
                                             name="rcpb")
                            nc.scalar.copy(rcp_b[:], rcp_ps[:])
                            nc.vector.tensor_mul(oav[b][:, h, il],
                                                 pav[h][:], rcp_b[:])

                        wo_exchange(b, t)
                        wo_pending.append((b, t))
                        # run the oldest wo block whose AllGather has had two
                        # chunks of attention to complete
                        if len(wo_pending) > 2:
                            wb_, wt_ = wo_pending.pop(0)
                            wo_block(wb_, wt_, psS)
                    if b == B - 1:
                        while wo_pending:
                            wb_, wt_ = wo_pending.pop(0)
                            wo_block(wb_, wt_, psS)

    nc.compile()
    return nc


def _prep_inputs(x, freqs_cos, freqs_sin, wq, wk, wv, wo):
    x = np.asarray(x, np.float32).reshape(NS, DIM)
    xT = np.ascontiguousarray(
        x.T.reshape(MB, P, 8, 512).transpose(0, 2, 1, 3)).astype(BF)
    cos = np.asarray(freqs_cos, np.float32)
    sin = np.asarray(freqs_sin, np.float32)
    cosT = np.ascontiguousarray(np.tile(cos, (B, 1)).T).astype(BF)
    sinT = np.ascontiguousarray(np.tile(sin, (B, 1)).T).astype(BF)

    perm = np.r_[np.arange(0, HD, 2), np.arange(1, HD, 2)]
    scale = np.float32(1.0 / np.sqrt(HD))
    wq = np.asarray(wq, np.float32) * scale
    wk = np.asarray(wk, np.float32)
    wv = np.asarray(wv, np.float32)
    wo = np.asarray(wo, np.float32)

    masks = np.zeros((P, 4, 512), np.float32)
    for p in range(4):
        for isub in range(4):
            sl = slice(isub * 128, (isub + 1) * 128)
            if p < isub:
                masks[:, p, sl] = 1.0
            elif p == isub:
                masks[:, p, sl] = np.triu(np.ones((P, P), np.float32))
    maskb = np.ascontiguousarray(
        (-30.0 * (1.0 - masks)).reshape(P, 4 * 512)).astype(BF)

    def tile_w(w):
        # (2048, d) -> (128, 16*d): row mi holds [mb, d] contiguously
        d = w.shape[1]
        return np.ascontiguousarray(
            w.reshape(MB, P, d).transpose(1, 0, 2).reshape(P, MB * d)).astype(BF)

    in_maps = []
    for c in range(NC):
        wq_c = wq[:, c * 256:(c + 1) * 256]
        wq_cp = np.concatenate([wq_c[:, h * HD + perm] for h in range(2)], axis=1)
        in_maps.append({
            "xT": xT,
            "wq_c": tile_w(wq_cp),
            "wk_c": tile_w(wk[:, c * HD:(c + 1) * HD][:, perm]),
            "wv_c": tile_w(wv[:, c * HD:(c + 1) * HD]),
            "wo_c": tile_w(wo[:, c * 256:(c + 1) * 256]),
            "cosT": cosT,
            "sinT": sinT,
            "maskb": maskb,
        })
    return in_maps


def _run(inputs, trace=False, **kw):
    from concourse.bass_utils import run_bass_kernel_spmd

    if "nc" not in _cache:
        _cache["nc"] = _build()
    nc = _cache["nc"]
    in_maps = _prep_inputs(**inputs)
    res = run_bass_kernel_spmd(
        nc, in_maps, core_ids=list(range(NC)), trace=trace, **kw
    )
    out = np.empty((NS, DIM), np.float32)
    for c in range(NC):
        out[:, c * 256:(c + 1) * 256] = res.results[c]["outT"].T
    return out.reshape(B, S, DIM), res


def kernel(**inputs) -> np.ndarray:
    out, _ = _run(inputs, trace=False)
    return out
